# revision 26
# baseline (speedup 1.0000x reference)
"""Trainium2 Bass kernel for an additive-attention layer.

Reference math (per batch b):
    q_proj = query @ W1 + b1                       # [U]
    v_proj = values[b] @ W2 + b2                   # [T, U]
    score  = tanh(q_proj + v_proj) @ V + bV        # [T, 1]
    aw     = softmax(score, axis=T)
    ctx    = sum_t aw[t] * values[b, t]            # [D]
    returns (ctx [B, D], aw [B, T, 1])

Sharding: data-parallel over batch B=64 across 8 cores (8 batches/core).
Weights are tiny and replicated; q_proj (+ b1 + b2) is folded into a
per-(batch, u) tanh bias computed on host (0.1% of the FLOPs).  bV shifts
all scores of a batch equally and is softmax-invariant, so it drops out.

Per-core dataflow (B_L=8, T=4096, D=U=256):
  - stream values fp32 from HBM once (33.5 MB), convert to a resident
    bf16 copy in SBUF (natural [t, d] layout, feeds the context matmul)
  - PE-transpose each [128t, 128d] bf16 tile -> vT [d, t] (PSUM bf16),
    copy to SBUF; projection matmul with W2 quadrants stationary
    -> v_projT [u, t] in PSUM fp32
  - tanh on ScalarE with per-partition bias = q_projT column -> SBUF bf16
  - score matmul: tanh tile [u, t] stationary, V [u, 1] moving -> one fp32
    column per 128 timesteps in a persistent PSUM scoreboard [128, B_L*T/128]
  - exp on ScalarE (scores are O(+-4): no max subtraction needed) with fused
    free-dim accumulation; cross-partition sum via ones-matmul
  - context matmul: exp-weight column [128t, 1] stationary, natural bf16
    values [128t, 256d] moving, accumulated over T in PSUM
  - unnormalized exp weights / context and the per-batch sums are DMA'd out;
    the host does the final divide (O(B*T) scalar work).
"""

import numpy as np
import ml_dtypes

import concourse.bass as bass
import concourse.bacc as bacc
import concourse.tile as tile
import concourse.mybir as mybir
from concourse.bass_utils import run_bass_kernel_spmd

F32 = mybir.dt.float32
BF16 = mybir.dt.bfloat16
AF = mybir.ActivationFunctionType

N_CORES = 8
B_GLOBAL, T_FULL, D, U = 64, 4096, 256, 256
P = 128

# test-harness hooks (kernel() behaves identically with these defaults)
TRACE = False
LAST_RESULT = None

# load path: True = SWDGE cast-DMA (fp32 HBM -> bf16 SBUF in one step);
# False = HWDGE fp32 load + DVE convert
CAST_DMA = True


def build_program(BL=B_GLOBAL // N_CORES, T=T_FULL, reps=1, ablate=frozenset()):
    """Emit the per-core Bass program. Parameterized so tests can build a
    small config for fast CoreSim iteration; reps>1 wraps the whole body
    in a hardware loop (the body is idempotent) for amortized timing."""
    TB = T // 512            # 512-timestep blocks
    NCOL = BL * TB * 4       # one fp32 score column per 128 timesteps
    assert T % 512 == 0 and NCOL <= 512

    nc = bacc.Bacc("TRN2", target_bir_lowering=False, debug=False)

    values_d = nc.dram_tensor("values", [BL, T, D], F32, kind="ExternalInput")
    qb_d = nc.dram_tensor("qb", [P, 2, BL], F32, kind="ExternalInput")
    w2_d = nc.dram_tensor("w2b", [P, 2, U], BF16, kind="ExternalInput")
    v_d = nc.dram_tensor("vb", [P, 2], BF16, kind="ExternalInput")
    idb_d = nc.dram_tensor("idb", [P, P], BF16, kind="ExternalInput")
    onescol_d = nc.dram_tensor("onescol", [P, 1], F32, kind="ExternalInput")

    # unnormalized context rows, exp weights (partition-major), per-batch sums
    ctx_d = nc.dram_tensor("ctxr", [BL, P, 2], F32, kind="ExternalOutput")
    attn_d = nc.dram_tensor("attnr", [BL, P, TB * 4], F32, kind="ExternalOutput")
    ssum_d = nc.dram_tensor("ssum", [1, BL], F32, kind="ExternalOutput")

    with tile.TileContext(nc) as tc:
        with (
            tc.tile_pool(name="const", bufs=1) as cpool,
            tc.tile_pool(name="res", bufs=1) as respool,
            tc.tile_pool(name="load", bufs=4) as loadpool,
            tc.tile_pool(name="vt", bufs=3) as vtpool,
            tc.tile_pool(name="th", bufs=3) as thpool,
            tc.tile_pool(name="smx", bufs=1) as smxpool,
            tc.tile_pool(name="cxo", bufs=2) as cxopool,
            tc.tile_pool(name="ptp", bufs=2, space="PSUM") as tppool,
            tc.tile_pool(name="pvp", bufs=3, space="PSUM") as vppool,
            tc.tile_pool(name="psc", bufs=1, space="PSUM") as scpool,
            tc.tile_pool(name="pcx", bufs=1, space="PSUM") as cxpool,
            tc.tile_pool(name="pms", bufs=1, space="PSUM") as mspool,
        ):
            # ---- constants ----
            w2_sb = cpool.tile([P, 2, U], BF16)
            nc.sync.dma_start(w2_sb[:], w2_d[:])
            v_sb = cpool.tile([P, 2], BF16)
            nc.sync.dma_start(v_sb[:], v_d[:])
            qb_sb = cpool.tile([P, 2, BL], F32)
            nc.sync.dma_start(qb_sb[:], qb_d[:])
            idb_sb = cpool.tile([P, P], BF16)
            nc.sync.dma_start(idb_sb[:], idb_d[:])
            onescol_sb = cpool.tile([P, 1], F32)
            nc.sync.dma_start(onescol_sb[:], onescol_d[:])

            # ACT warm-up: absorb the qb DMA dependency + activation table
            # load on dummy ops so steady-state tanh/exp carry a single wait
            warm = cpool.tile([P, 1], F32)
            nc.scalar.activation(warm[:], qb_sb[:, 0, 0:1], AF.Tanh)
            nc.scalar.activation(warm[:], warm[:], AF.Exp)

            # ---- persistent state ----
            vnat = respool.tile([P, BL, TB, 4, D], BF16)     # resident bf16 values
            exp_sb = smxpool.tile([P, NCOL], F32)            # exp(score)
            wexp_sb = smxpool.tile([P, NCOL], BF16)          # bf16 copy for matmul
            sums_sb = smxpool.tile([P, BL], F32)             # per-partition exp sums
            ssum_sb = smxpool.tile([1, BL], F32)

            ps_sc = scpool.tile([P, NCOL], F32)              # score board

            import contextlib
            repctx = tc.For_i(0, reps, 1) if reps > 1 else contextlib.nullcontext()
            with repctx:
                emit_body(nc, tc, BL, TB, NCOL, vnat, exp_sb, wexp_sb, sums_sb,
                          ssum_sb, ps_sc, values_d, ctx_d, attn_d, ssum_d,
                          w2_sb, v_sb, qb_sb, idb_sb, onescol_sb,
                          loadpool, vtpool, thpool, cxopool,
                          tppool, vppool, cxpool, mspool, ablate)

    nc.compile()
    return nc


def emit_body(nc, tc, BL, TB, NCOL, vnat, exp_sb, wexp_sb, sums_sb,
              ssum_sb, ps_sc, values_d, ctx_d, attn_d, ssum_d,
              w2_sb, v_sb, qb_sb, idb_sb, onescol_sb,
              loadpool, vtpool, thpool, cxopool,
              tppool, vppool, cxpool, mspool, ablate=frozenset()):
            for b in range(BL):
                for t in range(TB):
                    src = values_d[b, t * 512:(t + 1) * 512, :].rearrange(
                        "(j p) d -> p j d", p=P
                    )
                    if CAST_DMA:
                        # SWDGE cast-DMA: fp32 HBM -> resident bf16 SBUF
                        nc.gpsimd.dma_start(vnat[:, b, t, :, :], src)
                    else:
                        vl = loadpool.tile([P, 4, D], F32)
                        nc.sync.dma_start(vl[:], src)
                        nc.vector.tensor_copy(vnat[:, b, t, :, :], vl[:])

                    # transpose 8x [128, 128] -> vT [d, (dh), t]; all eight
                    # land in one PSUM bank so a single DVE copy drains them
                    vt = vtpool.tile([P, 2, 512], BF16)
                    if "transpose" not in ablate:
                        tp = tppool.tile([P, 2, 4, P], BF16, name="tp")
                        for j in range(4):
                            for dh in range(2):
                                nc.tensor.transpose(
                                    tp[:, dh, j, :],
                                    vnat[:, b, t, j, dh * P:(dh + 1) * P],
                                    idb_sb[:],
                                )
                        nc.vector.tensor_copy(vt[:], tp[:])

                    # projection + tanh, per u-half
                    th = thpool.tile([P, 2, 512], BF16)
                    for uh in range(2 if "vproj" not in ablate else 0):
                        vp = vppool.tile([P, 512], F32, name="vp")
                        for dh in range(2):
                            nc.tensor.matmul(
                                vp[:],
                                w2_sb[:, dh, uh * P:(uh + 1) * P],
                                vt[:, dh, :],
                                start=(dh == 0),
                                stop=(dh == 1),
                            )
                        nc.scalar.activation(
                            th[:, uh, :], vp[:], AF.Tanh, bias=qb_sb[:, uh, b:b + 1]
                        )

                    # score: tanh tiles stationary, V moving -> [128t, 1] columns
                    for j in range(4 if "score" not in ablate else 0):
                        col = (b * TB + t) * 4 + j
                        for uh in range(2):
                            nc.tensor.matmul(
                                ps_sc[:, col:col + 1],
                                th[:, uh, j * P:(j + 1) * P],
                                v_sb[:, uh:uh + 1],
                                start=(uh == 0),
                                stop=(uh == 1),
                            )

                # batch b scores complete: exp (+ fused partial sums), bf16 copy
                if "softmax" in ablate:
                    continue
                c0, c1 = b * TB * 4, (b + 1) * TB * 4
                nc.scalar.activation(
                    exp_sb[:, c0:c1], ps_sc[:, c0:c1], AF.Exp,
                    accum_out=sums_sb[:, b:b + 1],
                )
                nc.vector.tensor_copy(wexp_sb[:, c0:c1], exp_sb[:, c0:c1])
                # unnormalized weights out (partition-major; host transposes)
                nc.sync.dma_start(attn_d[b], exp_sb[:, c0:c1])

                # context accumulation for batch b (unnormalized, transposed):
                # values tile stationary, weight column moving ->
                # ctxT [128 d_in, dh] per batch; host re-interleaves
                nlast = TB * 4 - 1
                if "ctx" in ablate:
                    continue
                ps_cx = cxpool.tile([P, 2], F32, name="pscx")
                for t in range(TB):
                    for j in range(4):
                        col = (b * TB + t) * 4 + j
                        k = t * 4 + j
                        for dh in range(2):
                            nc.tensor.matmul(
                                ps_cx[:, dh:dh + 1],
                                vnat[:, b, t, j, dh * P:(dh + 1) * P],
                                wexp_sb[:, col:col + 1],
                                start=(k == 0 and dh == 0),
                                stop=(k == nlast and dh == 1),
                            )
                cxo = cxopool.tile([P, 2], F32, name="cxo")
                nc.vector.tensor_copy(cxo[:], ps_cx[:])
                nc.sync.dma_start(ctx_d[b], cxo[:])

            # ---- cross-partition sum of exp sums ----
            if "softmax" in ablate:
                return
            ps_sums = mspool.tile([1, BL], F32)
            nc.tensor.matmul(ps_sums[:], onescol_sb[:], sums_sb[:],
                             start=True, stop=True)
            nc.vector.tensor_copy(ssum_sb[:], ps_sums[:])
            nc.sync.dma_start(ssum_d[:], ssum_sb[:])


def _host_prep(W2, V):
    """Host-side tiny-tensor prep shared by all cores."""
    w2b = (
        W2.astype(ml_dtypes.bfloat16)
        .reshape(2, P, U)
        .transpose(1, 0, 2)
        .copy()
    )
    vb = V.astype(ml_dtypes.bfloat16)[:, 0].reshape(2, P).T.copy()
    idb = np.eye(P, dtype=ml_dtypes.bfloat16)
    onescol = np.ones((P, 1), np.float32)
    return w2b, vb, idb, onescol


def make_in_maps(query, values, W1, b1, W2, b2, V):
    """Per-core input maps (host-side prep is all O(KB) weight work)."""
    BL = B_GLOBAL // N_CORES
    qb_all = query @ W1 + b1 + b2                          # [B, U] fp32
    w2b, vb, idb, onescol = _host_prep(W2, V)

    in_maps = []
    for c in range(N_CORES):
        qb = qb_all[c * BL:(c + 1) * BL]                   # [BL, U]
        qbt = (
            np.ascontiguousarray(qb.T)                     # [U, BL]
            .reshape(2, P, BL)
            .transpose(1, 0, 2)
            .copy()
        )
        in_maps.append(
            {
                "values": values[c * BL:(c + 1) * BL],
                "qb": qbt,
                "w2b": w2b,
                "vb": vb,
                "idb": idb,
                "onescol": onescol,
            }
        )
    return in_maps


def kernel(query, values, W1, b1, W2, b2, V, bV, **_):
    query = np.ascontiguousarray(np.asarray(query, np.float32))
    values = np.ascontiguousarray(np.asarray(values, np.float32))
    W1 = np.asarray(W1, np.float32)
    b1 = np.asarray(b1, np.float32)
    W2 = np.asarray(W2, np.float32)
    b2 = np.asarray(b2, np.float32)
    V = np.asarray(V, np.float32)

    BL = B_GLOBAL // N_CORES
    nc = build_program(BL=BL, T=T_FULL)
    in_maps = make_in_maps(query, values, W1, b1, W2, b2, V)

    res = run_bass_kernel_spmd(
        nc, in_maps, core_ids=list(range(N_CORES)), trace=TRACE
    )
    globals()["LAST_RESULT"] = res

    ctxs, attns = [], []
    for c in range(N_CORES):
        r = res.results[c]
        ssum = np.asarray(r["ssum"], np.float32)[0]        # [BL]
        # ctxr [BL, P, 2]: d = dh*128 + p
        cx = np.asarray(r["ctxr"], np.float32).transpose(0, 2, 1).reshape(BL, D)
        ctxs.append(cx / ssum[:, None])
        # attnr [BL, P, TB*4]: t = col*128 + p  ->  [BL, T]
        aw = np.asarray(r["attnr"], np.float32).transpose(0, 2, 1).reshape(BL, T_FULL)
        attns.append(aw / ssum[:, None])
    ctx = np.concatenate(ctxs, axis=0)
    attn = np.concatenate(attns, axis=0)[:, :, None]
    return ctx.astype(np.float32), attn.astype(np.float32)
